# revision 57
# baseline (speedup 1.0000x reference)
"""Multi-head attention (B=8, N=1024, H=12, D=64, C=768) on 8 trn2 cores.

Sharding: data-parallel over batch. Core b computes attention for x[b];
weights are replicated. No collectives.

Per-core dataflow:
  gen:   qkT[t][128ch, 1024n] and v_sb[m][128n, 768ch] via fp8e4
         DoubleRow matmuls with 3-term hi/lo error compensation
         (hi@hi + hi@lo + lo@hi); host prescales x by 4 and W_qkv by 32 to
         keep fp8 out of subnormals, the 128x output scale is folded into
         the exp scale and W_proj. K=256 per DR step at 0.5 cycles/row ->
         4.5N per 768-contraction vs 6N for bf16. Evictions to bf16.
  attn per (pair t, nh half, m chunk):
         s_ps[128m, 1024] = [j0 | j1] S^T chunks (bf16, K=64 row-tiled)
         p = exp(s * 0.125/16384)  (ACT, bf16 out)  [the ACT chain, ~1us each]
  PV (per nh, one psum accumulation group per 2KB bank at a time, so each
      64-col accumulator runs its full m-contraction consecutively):
         out[n, d] += p_block[128m,128n].T @ v[128m,64]
           (out free dim = 64 -> half the PE cost of the v-stationary form)
         den[n] += p_block.T @ ones  (N=1 matmuls, own psum ring slot)
  norm:  h_n[n, 128hd] = h_raw * (1/den)   (DVE tensor_scalar per head)
  trans: hT[128hd, n] via PE transpose (identity matmul), evict bf16
  proj:  y[m-chunk] = hT.T @ W_proj  (6 bf16 k-steps, fp32 psum, bf16 out;
         m-chunks 0..3 overlap the tail of the exp chain)

The emission order software-pipelines chunks (PV trails S/exp by `lag`) so
the ACT exp chain runs continuously while PE fills slack with gen/transpose
work. Input loads are consolidated into few large strided DMAs ordered so
the first S chunk only waits for x plus the pair-0 weight columns.
"""
from collections import deque

import numpy as np

import concourse.bass as bass
import concourse.mybir as mybir
import concourse.tile as tile
from concourse import bacc
from concourse.bass_utils import run_bass_kernel_spmd

BF16 = mybir.dt.bfloat16
F32 = mybir.dt.float32
F8 = mybir.dt.float8e4
DR = mybir.MatmulPerfMode.DoubleRow

B, N, C = 8, 1024, 768
H, D = 12, 64
HID = H * D            # 768
KT = C // 128          # 6 feature k-tiles
MT = N // 128          # 8 sequence m-tiles
SCALE = D ** -0.5      # 0.125

_cached_nc = None

DEFAULT_OPTS = dict(
    lag=4,            # PV trails S/exp by this many chunks
    pt_bufs=34,       # p (exp output) ring
    hn_bufs=12,       # normalized-h ring
    den_delay=2,      # chunks between pv evict and den matmuls
    tr_delay=2,       # chunks into next pair before transposes
    pv0_slot=18,      # pair-0 nh0 PV batch deferred until v-gen lands
    pv01_off=5,       # pair-0 nh1 PV batch offset from pv0_slot
    end_offs=(0, 0, 1, 1, 3, 4, 5),
    mid_offs=(0, 2, 4, 6, 8, 9, 10),
    v0_base=2, v1_base=10, q1_base=7,
    q1_dues=(7, 8, 9, 10, 11, 12, 13, 14),
    warm_mm=0,        # dummy PE-ramp matmuls before the first quarter
    gen_evict_pool=0,  # route qk/v psum evictions to the idle GPSIMD engine
    y_split_dma=1,     # store y per-ph half instead of per-m
    pre_v=0,           # v-halves emitted before the first S chunk
    proj_split=0,      # m4-7 proj: k0-3 partials in pair-5 slack + finisher
    part_base=80,      # fill slot where the proj partials start
)


def build_program(**opts):
    o = dict(DEFAULT_OPTS, **opts)
    LAG = o["lag"]
    nc = bacc.Bacc(None, target_bir_lowering=False)

    # fp8 hi/lo pair-interleaved images, host-prepared in SBUF layout
    # [128p, 3 ksteps, 2 slots, cols]: feature = 256*s + 128*i + p.
    # x is prescaled by 4, W_qkv by 32 (keeps fp8 out of subnormals); the
    # resulting 128x output scale is folded into the exp scale and W_proj.
    xh_d = nc.dram_tensor("xh", [128, 3 * 2 * N], F8, kind="ExternalInput")
    xl_d = nc.dram_tensor("xl", [128, 3 * 2 * N], F8, kind="ExternalInput")
    whh_d = nc.dram_tensor("whh", [128, 3 * 2 * 1536], F8, kind="ExternalInput")
    whl_d = nc.dram_tensor("whl", [128, 3 * 2 * 1536], F8, kind="ExternalInput")
    wvh_d = nc.dram_tensor("wvh", [128, 3 * 2 * HID], F8, kind="ExternalInput")
    wvl_d = nc.dram_tensor("wvl", [128, 3 * 2 * HID], F8, kind="ExternalInput")
    wproj_d = nc.dram_tensor("wproj", [HID, C], BF16, kind="ExternalInput")
    ident_d = nc.dram_tensor("ident", [128, 129], BF16, kind="ExternalInput")
    y_d = nc.dram_tensor("y", [N, C], BF16, kind="ExternalOutput")

    with tile.TileContext(nc) as tc:
        with tc.tile_pool(name="persist", bufs=1) as persist, \
             tc.tile_pool(name="pt_pool", bufs=o["pt_bufs"]) as pt_pool, \
             tc.tile_pool(name="hraw_pool", bufs=3) as hraw_pool, \
             tc.tile_pool(name="hn_pool", bufs=o["hn_bufs"]) as hn_pool, \
             tc.tile_pool(name="r_pool", bufs=3) as r_pool, \
             tc.tile_pool(name="y_pool", bufs=3) as y_pool, \
             tc.tile_pool(name="ps_s", bufs=2, space="PSUM") as ps_s, \
             tc.tile_pool(name="ps_pv", bufs=2, space="PSUM") as ps_pv, \
             tc.tile_pool(name="ps_mm", bufs=2, space="PSUM") as ps_mm:

            # ---- resident tiles; loads consolidated into few strided DMAs,
            # emission order = DMA priority ----
            xh_t = persist.tile([128, 3 * 2 * N], F8, name="xh", tag="xh")
            xl_t = persist.tile([128, 3 * 2 * N], F8, name="xl", tag="xl")
            whh_t = persist.tile([128, 3 * 2 * 1536], F8, name="whh", tag="whh")
            whl_t = persist.tile([128, 3 * 2 * 1536], F8, name="whl", tag="whl")
            wvh_t = persist.tile([128, 3 * 2 * HID], F8, name="wvh", tag="wvh")
            wvl_t = persist.tile([128, 3 * 2 * HID], F8, name="wvl", tag="wvl")
            wp_all = persist.tile([128, KT * C], BF16, name="wp", tag="wp")
            ident = persist.tile([128, 129], BF16, name="ident", tag="ident")

            def x_sl(X, s, lo, hi):  # [128, 2, hi-lo] pair-slice of x image
                return X[:].rearrange("p (s i n) -> p s i n", s=3, i=2)[:, s, :, lo:hi]

            def w_sl(W, cols, s, lo, hi):
                return W[:].rearrange("p (s i c) -> p s i c", s=3, i=2)[:, s, :, lo:hi]

            def wp(k):
                return wp_all[:, k * C:(k + 1) * C]

            # first s-block of the x images + the pair-0 weight columns
            # (q tile 0 @ col 0, k tile 6 @ col 768) load first, so the first
            # qk quarter's k-loop starts as soon as each s-block lands
            SB = 2 * N  # one s-block of an x image
            nc.sync.dma_start(xh_t[:, :SB], xh_d[:, :SB])
            nc.sync.dma_start(xl_t[:, :SB], xl_d[:, :SB])
            whh_o = whh_t[:].rearrange("p (s i c) -> p s i c", s=3, i=2)
            whh_i = whh_d[:, :].rearrange("p (s i c) -> p s i c", s=3, i=2)
            whl_o = whl_t[:].rearrange("p (s i c) -> p s i c", s=3, i=2)
            whl_i = whl_d[:, :].rearrange("p (s i c) -> p s i c", s=3, i=2)
            for lo, hi in ((0, 128), (768, 896)):
                nc.sync.dma_start(whh_o[:, :, :, lo:hi], whh_i[:, :, :, lo:hi])
                nc.sync.dma_start(whl_o[:, :, :, lo:hi], whl_i[:, :, :, lo:hi])
            nc.sync.dma_start(xh_t[:, SB:], xh_d[:, SB:])
            nc.sync.dma_start(xl_t[:, SB:], xl_d[:, SB:])
            nc.sync.dma_start(wvh_t[:], wvh_d[:, :])
            nc.sync.dma_start(wvl_t[:], wvl_d[:, :])
            for lo, hi in ((128, 768), (896, 1536)):
                nc.sync.dma_start(whh_o[:, :, :, lo:hi], whh_i[:, :, :, lo:hi])
                nc.sync.dma_start(whl_o[:, :, :, lo:hi], whl_i[:, :, :, lo:hi])
            nc.sync.dma_start(ident[:], ident_d[:, :])
            ones = ident[:, 128:129]

            # warm the exp table during the DMA prefix
            warm = persist.tile([1, 8], F32, name="warm", tag="warm")
            nc.gpsimd.memset(warm[:], 0.0)
            nc.scalar.activation(warm[:], warm[:],
                                 mybir.ActivationFunctionType.Exp)
            # dummy matmuls ramp the PE p-state out of half-clock while the
            # input DMAs are still in flight, so the real qk generation runs
            # at full clock from its first instruction
            if o["warm_mm"]:
                warm_w = persist.tile([128, 128], BF16, name="warm_w", tag="warm_w")
                nc.vector.memset(warm_w[:], 0.0)
                wps = ps_mm.tile([128, 128], F32, name="warm_ps", tag="mm")
                for _ in range(o["warm_mm"]):
                    nc.tensor.matmul(wps[:], warm_w[:], warm_w[:],
                                     start=True, stop=True)

            qkT = [persist.tile([128, N], BF16, name=f"qkT{t}", tag=f"qkT{t}")
                   for t in range(12)]
            v_sb = [persist.tile([128, HID], BF16, name=f"vsb{m}", tag=f"vsb{m}")
                    for m in range(MT)]
            hT = qkT[:6]  # transposed outputs overwrite the dead Q tiles

            # ---- emission helpers ----
            # fp8 DoubleRow 3-term compensated GEMMs: hi@hi + hi@lo + lo@hi
            def qk_quarter(t, q):
                ps = ps_mm.tile([128, 256], F32, name="ps_qk", tag="mm")
                n9 = 0
                for s in range(3):
                    for Wt, Xt in ((whh_t, xh_t), (whl_t, xh_t), (whh_t, xl_t)):
                        nc.tensor.matmul(
                            ps[:],
                            w_sl(Wt, 1536, s, t * 128, (t + 1) * 128),
                            x_sl(Xt, s, q * 256, (q + 1) * 256),
                            start=(n9 == 0), stop=(n9 == 8), perf_mode=DR)
                        n9 += 1
                eng = nc.gpsimd if o["gen_evict_pool"] else nc.vector
                eng.tensor_copy(qkT[t][:, q * 256:(q + 1) * 256], ps[:])

            def v_half(m, vh):
                ps = ps_mm.tile([128, 384], F32, name="ps_v", tag="mm")
                n9 = 0
                for s in range(3):
                    for Xt, Wt in ((xh_t, wvh_t), (xh_t, wvl_t), (xl_t, wvh_t)):
                        nc.tensor.matmul(
                            ps[:],
                            x_sl(Xt, s, m * 128, (m + 1) * 128),
                            w_sl(Wt, HID, s, vh * 384, (vh + 1) * 384),
                            start=(n9 == 0), stop=(n9 == 8), perf_mode=DR)
                        n9 += 1
                eng = nc.gpsimd if o["gen_evict_pool"] else nc.vector
                eng.tensor_copy(v_sb[m][:, vh * 384:(vh + 1) * 384], ps[:])

            # state per in-flight chunk, keyed by global chunk index
            p_of = {}       # i -> p tile handle
            pv_of = {}      # (t, nh) -> pv psum tile handle
            hraw_of = {}    # (t, nh) -> raw h tile
            hn_of = {}      # (t, nh, c) -> normalized h tile

            def S_exp(i, t, nh, m):
                qT_t, kT_t = qkT[t], qkT[6 + t]
                msl = slice(m * 128, (m + 1) * 128)
                nsl = slice(nh * 512, (nh + 1) * 512)
                s = ps_s.tile([128, 1024], F32, name="s_ps", tag="s")
                for j in range(2):
                    psl = slice(j * 64, (j + 1) * 64)
                    nc.tensor.matmul(s[:, j * 512:(j + 1) * 512],
                                     kT_t[psl, msl], qT_t[psl, nsl],
                                     start=True, stop=True)
                p = pt_pool.tile([128, 1024], BF16, name="p_sb", tag="p")
                # q,k carry the 128x fp8 prescale each -> S is 16384x
                nc.scalar.activation(p[:], s[:],
                                     mybir.ActivationFunctionType.Exp,
                                     scale=SCALE / 16384.0)
                p_of[i] = p

            def pv_piece(t, nh, j, cs, mhi=MT):
                # one psum accumulation group may be pending per bank, so each
                # 64-col accumulator runs its full m-contraction consecutively;
                # pieces are spread over chunk slots to avoid starving ACT.
                # For the last pair the group closes at m6 and the m7
                # contribution is added late with start=False (a plain psum
                # accumulate on hardware), so most of the batch overlaps the
                # exp chain.
                if (t, nh) not in pv_of:
                    pv_of[(t, nh)] = (
                        ps_pv.tile([128, 512], F32, name="pv_ps", tag="pv"),
                        ps_pv.tile([128, 8], F32, name="den_ps", tag="pv"))
                pv, den = pv_of[(t, nh)]
                h = 2 * t + j
                for c in cs:
                    a = j * 4 + c
                    for m in range(mhi):
                        i = (t * 2 + nh) * MT + m
                        nc.tensor.matmul(
                            pv[:, a * 64:(a + 1) * 64],
                            p_of[i][:, j * 512 + c * 128:j * 512 + (c + 1) * 128],
                            v_sb[m][:, h * 64:(h + 1) * 64],
                            start=(m == 0), stop=(m == mhi - 1))

            def den_finish(t, nh, mhi=MT):
                pv, den = pv_of[(t, nh)]
                for j in range(2):
                    for c in range(4):
                        a = j * 4 + c
                        for m in range(mhi):
                            i = (t * 2 + nh) * MT + m
                            nc.tensor.matmul(
                                den[:, a:a + 1],
                                p_of[i][:, j * 512 + c * 128:j * 512 + (c + 1) * 128],
                                ones,
                                start=(m == 0), stop=(m == mhi - 1))
                if mhi == MT:
                    evict_recip(t, nh)

            def evict_recip(t, nh):
                pv, den = pv_of[(t, nh)]
                hraw = hraw_pool.tile([128, 512], BF16, name="hraw", tag="hraw")
                nc.vector.tensor_copy(hraw[:], pv[:])
                r = r_pool.tile([128, 8], F32, name="r_sb", tag="r")
                nc.vector.reciprocal(r[:], den[:])
                hraw_of[(t, nh)] = (hraw, r)
                pv_of.pop((t, nh))

            def den_norm(t, nh):
                hraw, r = hraw_of.pop((t, nh))
                for c in range(4):
                    hn = hn_pool.tile([128, 128], BF16, name="hn", tag="hn")
                    for j in range(2):
                        a = j * 4 + c
                        nc.vector.tensor_scalar_mul(
                            hn[:, j * 64:(j + 1) * 64],
                            hraw[:, a * 64:(a + 1) * 64],
                            r[:, a:a + 1])
                    hn_of[(t, nh, c)] = hn

            def tr_nh(t, nh):
                for c in range(4):
                    hn = hn_of.pop((t, nh, c))
                    trp = ps_mm.tile([128, 128], BF16, name="tr_ps", tag="mm")
                    nc.tensor.transpose(trp[:], hn[:], ident[:, 0:128])
                    nc.vector.tensor_copy(
                        hT[t][:, nh * 512 + c * 128:nh * 512 + (c + 1) * 128],
                        trp[:])

            PROJ_POOLS = [(ps_mm, "mm"), (ps_s, "s"), (ps_pv, "pv")]
            y_parts = {}

            def proj_partial(m, ph):
                # k0..3 contribution of y[m-chunk, ph-half], computed during
                # pair-5 slack (needs only pairs 0..3), evicted to bf16
                ps = ps_mm.tile([128, 384], F32, name="ps_yp", tag="mm")
                for k in range(4):
                    nc.tensor.matmul(ps[:], hT[k][:, m * 128:(m + 1) * 128],
                                     wp(k)[:, ph * 384:(ph + 1) * 384],
                                     start=(k == 0), stop=(k == 3))
                yp = y_pool.tile([128, 384], BF16, name="y_part", tag="ypart", bufs=9)
                nc.vector.tensor_copy(yp[:], ps[:])
                y_parts[(m, ph)] = yp

            def proj_fin(m):
                # k4+k5 finisher + recombine with the k0..3 partial
                y_sb = y_pool.tile([128, C], BF16, name="y_sb", tag="y")
                for ph in range(2):
                    ps = ps_mm.tile([128, 384], F32, name="ps_yf", tag="mm")
                    for k in (4, 5):
                        nc.tensor.matmul(ps[:], hT[k][:, m * 128:(m + 1) * 128],
                                         wp(k)[:, ph * 384:(ph + 1) * 384],
                                         start=(k == 4), stop=(k == 5))
                    dst = y_sb[:, ph * 384:(ph + 1) * 384]
                    nc.vector.tensor_tensor(dst, ps[:], y_parts.pop((m, ph))[:],
                                            mybir.AluOpType.add)
                    nc.sync.dma_start(
                        y_d[m * 128:(m + 1) * 128,
                            ph * 384:(ph + 1) * 384], dst)

            def proj(m):
                y_sb = y_pool.tile([128, C], BF16, name="y_sb", tag="y")
                for ph in range(2):
                    pool, tg = PROJ_POOLS[(2 * m + ph) % 3]
                    ps = pool.tile([128, 384], F32, name="ps_y", tag=tg)
                    for k in range(KT):
                        nc.tensor.matmul(ps[:], hT[k][:, m * 128:(m + 1) * 128],
                                         wp(k)[:, ph * 384:(ph + 1) * 384],
                                         start=(k == 0), stop=(k == KT - 1))
                    dst = y_sb[:, ph * 384:(ph + 1) * 384]
                    if ph == 0:
                        nc.scalar.copy(dst, ps[:])
                    else:
                        nc.vector.tensor_copy(dst, ps[:])
                    if o["y_split_dma"]:
                        nc.sync.dma_start(
                            y_d[m * 128:(m + 1) * 128,
                                ph * 384:(ph + 1) * 384], dst)
                if not o["y_split_dma"]:
                    nc.sync.dma_start(y_d[m * 128:(m + 1) * 128, :], y_sb[:])

            # ---- schedule ----
            chunks = [(t, nh, m)
                      for t in range(6) for nh in range(2) for m in range(MT)]

            # deferred actions: (due_chunk_index, fn); fill work for PE slack
            events = []
            fills = deque()
            # remaining pair-0 qk quarters, interleaved with the first chunks
            for slot, (tt, q) in enumerate([(6, 1), (6, 2), (6, 3)]):
                fills.append((slot, lambda tt=tt, q=q: qk_quarter(tt, q)))
            fills.append((5, lambda: qk_quarter(0, 2)))
            fills.append((6, lambda: qk_quarter(0, 3)))
            # v tiles during pair 0, one half per slot (pair 0's PV batches
            # are deferred past them)
            for vi in range(o["pre_v"], 2 * MT):
                m, vh = vi // 2, vi % 2
                base = o["v0_base"] if vh == 0 else o["v1_base"]
                fills.append((base + m, lambda m=m, vh=vh: v_half(m, vh)))
            # qk tiles for pair t generated during pair t-1
            for tp in range(1, 6):
                qs = [(tp, q) for q in range(4)] + [(6 + tp, q) for q in range(4)]
                for idx, (tt, q) in enumerate(qs):
                    if tp == 1:
                        # deadline-staggered: q tiles 2,3 and late k quarters
                        # aren't read until pair-1 nh1 / later m-chunks, so
                        # they move past the v-gen-congested slots
                        due = o["q1_dues"][idx]
                    else:
                        due = (tp - 1) * 16 + 2 * idx
                    fills.append((due, lambda tt=tt, q=q: qk_quarter(tt, q)))
            fills = deque(sorted(fills, key=lambda f: f[0]))

            if o["proj_split"]:
                for idx, (m, ph) in enumerate(
                        [(m, ph) for m in range(4, MT) for ph in range(2)]):
                    fills.append((o["part_base"] + idx,
                                  lambda m=m, ph=ph: proj_partial(m, ph)))
            fills = deque(sorted(fills, key=lambda f: f[0]))

            # initial generation: just enough for the first S chunks
            qk_quarter(0, 0)
            qk_quarter(0, 1)
            qk_quarter(6, 0)
            for pv_i in range(o["pre_v"]):
                v_half(pv_i // 2, pv_i % 2)

            n_chunks = len(chunks)
            for i in range(n_chunks + LAG):
                if i < n_chunks:
                    t, nh, m = chunks[i]
                    if (t, nh, m) == (4, 0, 0):
                        nc.sync.dma_start(
                            wp_all[:].rearrange("p (k n) -> p k n", k=KT),
                            wproj_d[:, :].rearrange("(k p) n -> p k n", p=128))
                    S_exp(i, t, nh, m)
                j = i - LAG
                if j >= 0:
                    tj, nhj, mj = chunks[j]
                    if mj == MT - 1:
                        due = i
                        if (tj, nhj) == (0, 0):
                            due = max(i, o["pv0_slot"])
                        elif (tj, nhj) == (0, 1):
                            due = max(i, o["pv0_slot"] + o["pv01_off"])
                        offs = o["end_offs"] if tj == 5 else o["mid_offs"]
                        for off, (jj, cs) in zip(offs, [(0, [0, 1]), (0, [2, 3]),
                                                        (1, [0, 1]), (1, [2, 3])]):
                            events.append(
                                (due + off,
                                 lambda tj=tj, nhj=nhj, jj=jj, cs=cs:
                                     pv_piece(tj, nhj, jj, cs)))
                        events.append((due + offs[4],
                                       lambda tj=tj, nhj=nhj: den_finish(tj, nhj)))
                        events.append((due + offs[5],
                                       lambda tj=tj, nhj=nhj: den_norm(tj, nhj)))
                        if not (tj == 5 and nhj == 1):
                            events.append((due + offs[6],
                                           lambda tj=tj, nhj=nhj: tr_nh(tj, nhj)))
                        if (tj, nhj) == (5, 0):
                            # proj of m-chunks 0..3 only needs pair-5 nh0
                            for pm in range(4):
                                events.append((due + offs[6] + 1,
                                               lambda pm=pm: proj(pm)))

                # run due deferred actions, then due fill items
                for ev in [e for e in events if e[0] <= i]:
                    events.remove(ev)
                    ev[1]()
                while fills and fills[0][0] <= i:
                    fills.popleft()[1]()

            for ev in sorted(events, key=lambda e: e[0]):
                ev[1]()
            tr_nh(5, 1)
            for m in range(4, MT):
                if o["proj_split"]:
                    proj_fin(m)
                else:
                    proj(m)

    nc.compile()
    return nc


def _pair_image(a, cols):
    """[768, cols] fp32 -> fp8 hi/lo SBUF images [128, 3*2*cols]
    with feature = 256*s + 128*i + p."""
    import ml_dtypes
    f8 = ml_dtypes.float8_e4m3
    hi = a.astype(f8)
    lo = (a - hi.astype(np.float32)).astype(f8)
    out = []
    for img in (hi, lo):
        out.append(np.ascontiguousarray(
            img.reshape(3, 2, 128, cols).transpose(2, 0, 1, 3).reshape(
                128, 3 * 2 * cols)))
    return out


def _run(inputs, trace=False, trace_kwargs=None):
    global _cached_nc
    import ml_dtypes
    bf16 = ml_dtypes.bfloat16
    x = np.asarray(inputs["x"], dtype=np.float32)
    wqkv = np.asarray(inputs["W_qkv"], dtype=np.float32)
    wproj = (np.asarray(inputs["W_proj"], dtype=np.float32) / 128.0).astype(bf16)
    whh, whl = _pair_image(wqkv[:, :2 * HID] * 32.0, 1536)
    wvh, wvl = _pair_image(wqkv[:, 2 * HID:] * 32.0, HID)
    ident = np.zeros((128, 129), dtype=bf16)
    ident[:, :128] = np.eye(128, dtype=np.float32).astype(bf16)
    ident[:, 128] = bf16(1.0)

    if _cached_nc is None:
        _cached_nc = build_program()
    nc = _cached_nc

    in_maps = []
    for b in range(B):
        xh, xl = _pair_image(np.ascontiguousarray(x[b].T) * 4.0, N)
        in_maps.append({"xh": xh, "xl": xl, "whh": whh, "whl": whl,
                        "wvh": wvh, "wvl": wvl, "wproj": wproj,
                        "ident": ident})
    kwargs = {}
    if trace:
        kwargs["trace"] = True
        if trace_kwargs:
            kwargs.update(trace_kwargs)
    try:
        res = run_bass_kernel_spmd(nc, in_maps, core_ids=list(range(B)), **kwargs)
    except Exception:
        # transient axon/PJRT hiccups happen; one retry
        res = run_bass_kernel_spmd(nc, in_maps, core_ids=list(range(B)), **kwargs)
    out = np.stack([np.asarray(r["y"], dtype=np.float32) for r in res.results],
                   axis=0)
    return out, res


def kernel(**inputs):
    out, _ = _run(inputs)
    return out


# revision 58
# speedup vs baseline: 1.0025x; 1.0025x over previous
"""Multi-head attention (B=8, N=1024, H=12, D=64, C=768) on 8 trn2 cores.

Sharding: data-parallel over batch. Core b computes attention for x[b];
weights are replicated. No collectives.

Per-core dataflow:
  gen:   qkT[t][128ch, 1024n] and v_sb[m][128n, 768ch] via fp8e4
         DoubleRow matmuls with 3-term hi/lo error compensation
         (hi@hi + hi@lo + lo@hi); host prescales x by 4 and W_qkv by 32 to
         keep fp8 out of subnormals, the 128x output scale is folded into
         the exp scale and W_proj. K=256 per DR step at 0.5 cycles/row ->
         4.5N per 768-contraction vs 6N for bf16. Evictions to bf16.
  attn per (pair t, nh half, m chunk):
         s_ps[128m, 1024] = [j0 | j1] S^T chunks (bf16, K=64 row-tiled)
         p = exp(s * 0.125/16384)  (ACT, bf16 out)  [the ACT chain, ~1us each]
  PV (per nh, one psum accumulation group per 2KB bank at a time, so each
      64-col accumulator runs its full m-contraction consecutively):
         out[n, d] += p_block[128m,128n].T @ v[128m,64]
           (out free dim = 64 -> half the PE cost of the v-stationary form)
         den[n] += p_block.T @ ones  (N=1 matmuls, own psum ring slot)
  norm:  h_n[n, 128hd] = h_raw * (1/den)   (DVE tensor_scalar per head)
  trans: hT[128hd, n] via PE transpose (identity matmul), evict bf16
  proj:  y[m-chunk] = hT.T @ W_proj  (6 bf16 k-steps, fp32 psum, bf16 out;
         m-chunks 0..3 overlap the tail of the exp chain)

The emission order software-pipelines chunks (PV trails S/exp by `lag`) so
the ACT exp chain runs continuously while PE fills slack with gen/transpose
work. Input loads are consolidated into few large strided DMAs ordered so
the first S chunk only waits for x plus the pair-0 weight columns.
"""
from collections import deque

import numpy as np

import concourse.bass as bass
import concourse.mybir as mybir
import concourse.tile as tile
from concourse import bacc
from concourse.bass_utils import run_bass_kernel_spmd

BF16 = mybir.dt.bfloat16
F32 = mybir.dt.float32
F8 = mybir.dt.float8e4
DR = mybir.MatmulPerfMode.DoubleRow

B, N, C = 8, 1024, 768
H, D = 12, 64
HID = H * D            # 768
KT = C // 128          # 6 feature k-tiles
MT = N // 128          # 8 sequence m-tiles
SCALE = D ** -0.5      # 0.125

_cached_nc = None

DEFAULT_OPTS = dict(
    lag=5,            # PV trails S/exp by this many chunks
    pt_bufs=36,       # p (exp output) ring
    hn_bufs=12,       # normalized-h ring
    den_delay=2,      # chunks between pv evict and den matmuls
    tr_delay=2,       # chunks into next pair before transposes
    pv0_slot=18,      # pair-0 nh0 PV batch deferred until v-gen lands
    pv01_off=5,       # pair-0 nh1 PV batch offset from pv0_slot
    end_offs=(0, 0, 1, 1, 3, 4, 5),
    mid_offs=(0, 2, 4, 6, 8, 9, 10),
    v0_base=2, v1_base=10, q1_base=7,
    q1_dues=(7, 8, 9, 10, 11, 12, 13, 14),
    warm_mm=0,        # dummy PE-ramp matmuls before the first quarter
    gen_evict_pool=0,  # route qk/v psum evictions to the idle GPSIMD engine
    y_split_dma=1,     # store y per-ph half instead of per-m
    pre_v=0,           # v-halves emitted before the first S chunk
    proj_split=0,      # m4-7 proj: k0-3 partials in pair-5 slack + finisher
    part_base=80,      # fill slot where the proj partials start
)


def build_program(**opts):
    o = dict(DEFAULT_OPTS, **opts)
    LAG = o["lag"]
    nc = bacc.Bacc(None, target_bir_lowering=False)

    # fp8 hi/lo pair-interleaved images, host-prepared in SBUF layout
    # [128p, 3 ksteps, 2 slots, cols]: feature = 256*s + 128*i + p.
    # x is prescaled by 4, W_qkv by 32 (keeps fp8 out of subnormals); the
    # resulting 128x output scale is folded into the exp scale and W_proj.
    xh_d = nc.dram_tensor("xh", [128, 3 * 2 * N], F8, kind="ExternalInput")
    xl_d = nc.dram_tensor("xl", [128, 3 * 2 * N], F8, kind="ExternalInput")
    whh_d = nc.dram_tensor("whh", [128, 3 * 2 * 1536], F8, kind="ExternalInput")
    whl_d = nc.dram_tensor("whl", [128, 3 * 2 * 1536], F8, kind="ExternalInput")
    wvh_d = nc.dram_tensor("wvh", [128, 3 * 2 * HID], F8, kind="ExternalInput")
    wvl_d = nc.dram_tensor("wvl", [128, 3 * 2 * HID], F8, kind="ExternalInput")
    wproj_d = nc.dram_tensor("wproj", [HID, C], BF16, kind="ExternalInput")
    ident_d = nc.dram_tensor("ident", [128, 129], BF16, kind="ExternalInput")
    y_d = nc.dram_tensor("y", [N, C], BF16, kind="ExternalOutput")

    with tile.TileContext(nc) as tc:
        with tc.tile_pool(name="persist", bufs=1) as persist, \
             tc.tile_pool(name="pt_pool", bufs=o["pt_bufs"]) as pt_pool, \
             tc.tile_pool(name="hraw_pool", bufs=3) as hraw_pool, \
             tc.tile_pool(name="hn_pool", bufs=o["hn_bufs"]) as hn_pool, \
             tc.tile_pool(name="r_pool", bufs=3) as r_pool, \
             tc.tile_pool(name="y_pool", bufs=3) as y_pool, \
             tc.tile_pool(name="ps_s", bufs=2, space="PSUM") as ps_s, \
             tc.tile_pool(name="ps_pv", bufs=2, space="PSUM") as ps_pv, \
             tc.tile_pool(name="ps_mm", bufs=2, space="PSUM") as ps_mm:

            # ---- resident tiles; loads consolidated into few strided DMAs,
            # emission order = DMA priority ----
            xh_t = persist.tile([128, 3 * 2 * N], F8, name="xh", tag="xh")
            xl_t = persist.tile([128, 3 * 2 * N], F8, name="xl", tag="xl")
            whh_t = persist.tile([128, 3 * 2 * 1536], F8, name="whh", tag="whh")
            whl_t = persist.tile([128, 3 * 2 * 1536], F8, name="whl", tag="whl")
            wvh_t = persist.tile([128, 3 * 2 * HID], F8, name="wvh", tag="wvh")
            wvl_t = persist.tile([128, 3 * 2 * HID], F8, name="wvl", tag="wvl")
            wp_all = persist.tile([128, KT * C], BF16, name="wp", tag="wp")
            ident = persist.tile([128, 129], BF16, name="ident", tag="ident")

            def x_sl(X, s, lo, hi):  # [128, 2, hi-lo] pair-slice of x image
                return X[:].rearrange("p (s i n) -> p s i n", s=3, i=2)[:, s, :, lo:hi]

            def w_sl(W, cols, s, lo, hi):
                return W[:].rearrange("p (s i c) -> p s i c", s=3, i=2)[:, s, :, lo:hi]

            def wp(k):
                return wp_all[:, k * C:(k + 1) * C]

            # first s-block of the x images + the pair-0 weight columns
            # (q tile 0 @ col 0, k tile 6 @ col 768) load first, so the first
            # qk quarter's k-loop starts as soon as each s-block lands
            SB = 2 * N  # one s-block of an x image
            nc.sync.dma_start(xh_t[:, :SB], xh_d[:, :SB])
            nc.sync.dma_start(xl_t[:, :SB], xl_d[:, :SB])
            whh_o = whh_t[:].rearrange("p (s i c) -> p s i c", s=3, i=2)
            whh_i = whh_d[:, :].rearrange("p (s i c) -> p s i c", s=3, i=2)
            whl_o = whl_t[:].rearrange("p (s i c) -> p s i c", s=3, i=2)
            whl_i = whl_d[:, :].rearrange("p (s i c) -> p s i c", s=3, i=2)
            for lo, hi in ((0, 128), (768, 896)):
                nc.sync.dma_start(whh_o[:, :, :, lo:hi], whh_i[:, :, :, lo:hi])
                nc.sync.dma_start(whl_o[:, :, :, lo:hi], whl_i[:, :, :, lo:hi])
            nc.sync.dma_start(xh_t[:, SB:], xh_d[:, SB:])
            nc.sync.dma_start(xl_t[:, SB:], xl_d[:, SB:])
            nc.sync.dma_start(wvh_t[:], wvh_d[:, :])
            nc.sync.dma_start(wvl_t[:], wvl_d[:, :])
            for lo, hi in ((128, 768), (896, 1536)):
                nc.sync.dma_start(whh_o[:, :, :, lo:hi], whh_i[:, :, :, lo:hi])
                nc.sync.dma_start(whl_o[:, :, :, lo:hi], whl_i[:, :, :, lo:hi])
            nc.sync.dma_start(ident[:], ident_d[:, :])
            ones = ident[:, 128:129]

            # warm the exp table during the DMA prefix
            warm = persist.tile([1, 8], F32, name="warm", tag="warm")
            nc.gpsimd.memset(warm[:], 0.0)
            nc.scalar.activation(warm[:], warm[:],
                                 mybir.ActivationFunctionType.Exp)
            # dummy matmuls ramp the PE p-state out of half-clock while the
            # input DMAs are still in flight, so the real qk generation runs
            # at full clock from its first instruction
            if o["warm_mm"]:
                warm_w = persist.tile([128, 128], BF16, name="warm_w", tag="warm_w")
                nc.vector.memset(warm_w[:], 0.0)
                wps = ps_mm.tile([128, 128], F32, name="warm_ps", tag="mm")
                for _ in range(o["warm_mm"]):
                    nc.tensor.matmul(wps[:], warm_w[:], warm_w[:],
                                     start=True, stop=True)

            qkT = [persist.tile([128, N], BF16, name=f"qkT{t}", tag=f"qkT{t}")
                   for t in range(12)]
            v_sb = [persist.tile([128, HID], BF16, name=f"vsb{m}", tag=f"vsb{m}")
                    for m in range(MT)]
            hT = qkT[:6]  # transposed outputs overwrite the dead Q tiles

            # ---- emission helpers ----
            # fp8 DoubleRow 3-term compensated GEMMs: hi@hi + hi@lo + lo@hi
            def qk_quarter(t, q):
                ps = ps_mm.tile([128, 256], F32, name="ps_qk", tag="mm")
                n9 = 0
                for s in range(3):
                    for Wt, Xt in ((whh_t, xh_t), (whl_t, xh_t), (whh_t, xl_t)):
                        nc.tensor.matmul(
                            ps[:],
                            w_sl(Wt, 1536, s, t * 128, (t + 1) * 128),
                            x_sl(Xt, s, q * 256, (q + 1) * 256),
                            start=(n9 == 0), stop=(n9 == 8), perf_mode=DR)
                        n9 += 1
                eng = nc.gpsimd if o["gen_evict_pool"] else nc.vector
                eng.tensor_copy(qkT[t][:, q * 256:(q + 1) * 256], ps[:])

            def v_half(m, vh):
                ps = ps_mm.tile([128, 384], F32, name="ps_v", tag="mm")
                n9 = 0
                for s in range(3):
                    for Xt, Wt in ((xh_t, wvh_t), (xh_t, wvl_t), (xl_t, wvh_t)):
                        nc.tensor.matmul(
                            ps[:],
                            x_sl(Xt, s, m * 128, (m + 1) * 128),
                            w_sl(Wt, HID, s, vh * 384, (vh + 1) * 384),
                            start=(n9 == 0), stop=(n9 == 8), perf_mode=DR)
                        n9 += 1
                eng = nc.gpsimd if o["gen_evict_pool"] else nc.vector
                eng.tensor_copy(v_sb[m][:, vh * 384:(vh + 1) * 384], ps[:])

            # state per in-flight chunk, keyed by global chunk index
            p_of = {}       # i -> p tile handle
            pv_of = {}      # (t, nh) -> pv psum tile handle
            hraw_of = {}    # (t, nh) -> raw h tile
            hn_of = {}      # (t, nh, c) -> normalized h tile

            def S_exp(i, t, nh, m):
                qT_t, kT_t = qkT[t], qkT[6 + t]
                msl = slice(m * 128, (m + 1) * 128)
                nsl = slice(nh * 512, (nh + 1) * 512)
                s = ps_s.tile([128, 1024], F32, name="s_ps", tag="s")
                for j in range(2):
                    psl = slice(j * 64, (j + 1) * 64)
                    nc.tensor.matmul(s[:, j * 512:(j + 1) * 512],
                                     kT_t[psl, msl], qT_t[psl, nsl],
                                     start=True, stop=True)
                p = pt_pool.tile([128, 1024], BF16, name="p_sb", tag="p")
                # q,k carry the 128x fp8 prescale each -> S is 16384x
                nc.scalar.activation(p[:], s[:],
                                     mybir.ActivationFunctionType.Exp,
                                     scale=SCALE / 16384.0)
                p_of[i] = p

            def pv_piece(t, nh, j, cs, mhi=MT):
                # one psum accumulation group may be pending per bank, so each
                # 64-col accumulator runs its full m-contraction consecutively;
                # pieces are spread over chunk slots to avoid starving ACT.
                # For the last pair the group closes at m6 and the m7
                # contribution is added late with start=False (a plain psum
                # accumulate on hardware), so most of the batch overlaps the
                # exp chain.
                if (t, nh) not in pv_of:
                    pv_of[(t, nh)] = (
                        ps_pv.tile([128, 512], F32, name="pv_ps", tag="pv"),
                        ps_pv.tile([128, 8], F32, name="den_ps", tag="pv"))
                pv, den = pv_of[(t, nh)]
                h = 2 * t + j
                for c in cs:
                    a = j * 4 + c
                    for m in range(mhi):
                        i = (t * 2 + nh) * MT + m
                        nc.tensor.matmul(
                            pv[:, a * 64:(a + 1) * 64],
                            p_of[i][:, j * 512 + c * 128:j * 512 + (c + 1) * 128],
                            v_sb[m][:, h * 64:(h + 1) * 64],
                            start=(m == 0), stop=(m == mhi - 1))

            def den_finish(t, nh, mhi=MT):
                pv, den = pv_of[(t, nh)]
                for j in range(2):
                    for c in range(4):
                        a = j * 4 + c
                        for m in range(mhi):
                            i = (t * 2 + nh) * MT + m
                            nc.tensor.matmul(
                                den[:, a:a + 1],
                                p_of[i][:, j * 512 + c * 128:j * 512 + (c + 1) * 128],
                                ones,
                                start=(m == 0), stop=(m == mhi - 1))
                if mhi == MT:
                    evict_recip(t, nh)

            def evict_recip(t, nh):
                pv, den = pv_of[(t, nh)]
                hraw = hraw_pool.tile([128, 512], BF16, name="hraw", tag="hraw")
                nc.vector.tensor_copy(hraw[:], pv[:])
                r = r_pool.tile([128, 8], F32, name="r_sb", tag="r")
                nc.vector.reciprocal(r[:], den[:])
                hraw_of[(t, nh)] = (hraw, r)
                pv_of.pop((t, nh))

            def den_norm(t, nh):
                hraw, r = hraw_of.pop((t, nh))
                for c in range(4):
                    hn = hn_pool.tile([128, 128], BF16, name="hn", tag="hn")
                    for j in range(2):
                        a = j * 4 + c
                        nc.vector.tensor_scalar_mul(
                            hn[:, j * 64:(j + 1) * 64],
                            hraw[:, a * 64:(a + 1) * 64],
                            r[:, a:a + 1])
                    hn_of[(t, nh, c)] = hn

            def tr_nh(t, nh):
                for c in range(4):
                    hn = hn_of.pop((t, nh, c))
                    trp = ps_mm.tile([128, 128], BF16, name="tr_ps", tag="mm")
                    nc.tensor.transpose(trp[:], hn[:], ident[:, 0:128])
                    nc.vector.tensor_copy(
                        hT[t][:, nh * 512 + c * 128:nh * 512 + (c + 1) * 128],
                        trp[:])

            PROJ_POOLS = [(ps_mm, "mm"), (ps_s, "s"), (ps_pv, "pv")]
            y_parts = {}

            def proj_partial(m, ph):
                # k0..3 contribution of y[m-chunk, ph-half], computed during
                # pair-5 slack (needs only pairs 0..3), evicted to bf16
                ps = ps_mm.tile([128, 384], F32, name="ps_yp", tag="mm")
                for k in range(4):
                    nc.tensor.matmul(ps[:], hT[k][:, m * 128:(m + 1) * 128],
                                     wp(k)[:, ph * 384:(ph + 1) * 384],
                                     start=(k == 0), stop=(k == 3))
                yp = y_pool.tile([128, 384], BF16, name="y_part", tag="ypart", bufs=9)
                nc.vector.tensor_copy(yp[:], ps[:])
                y_parts[(m, ph)] = yp

            def proj_fin(m):
                # k4+k5 finisher + recombine with the k0..3 partial
                y_sb = y_pool.tile([128, C], BF16, name="y_sb", tag="y")
                for ph in range(2):
                    ps = ps_mm.tile([128, 384], F32, name="ps_yf", tag="mm")
                    for k in (4, 5):
                        nc.tensor.matmul(ps[:], hT[k][:, m * 128:(m + 1) * 128],
                                         wp(k)[:, ph * 384:(ph + 1) * 384],
                                         start=(k == 4), stop=(k == 5))
                    dst = y_sb[:, ph * 384:(ph + 1) * 384]
                    nc.vector.tensor_tensor(dst, ps[:], y_parts.pop((m, ph))[:],
                                            mybir.AluOpType.add)
                    nc.sync.dma_start(
                        y_d[m * 128:(m + 1) * 128,
                            ph * 384:(ph + 1) * 384], dst)

            def proj(m):
                y_sb = y_pool.tile([128, C], BF16, name="y_sb", tag="y")
                for ph in range(2):
                    pool, tg = PROJ_POOLS[(2 * m + ph) % 3]
                    ps = pool.tile([128, 384], F32, name="ps_y", tag=tg)
                    for k in range(KT):
                        nc.tensor.matmul(ps[:], hT[k][:, m * 128:(m + 1) * 128],
                                         wp(k)[:, ph * 384:(ph + 1) * 384],
                                         start=(k == 0), stop=(k == KT - 1))
                    dst = y_sb[:, ph * 384:(ph + 1) * 384]
                    if ph == 0:
                        nc.scalar.copy(dst, ps[:])
                    else:
                        nc.vector.tensor_copy(dst, ps[:])
                    if o["y_split_dma"]:
                        nc.sync.dma_start(
                            y_d[m * 128:(m + 1) * 128,
                                ph * 384:(ph + 1) * 384], dst)
                if not o["y_split_dma"]:
                    nc.sync.dma_start(y_d[m * 128:(m + 1) * 128, :], y_sb[:])

            # ---- schedule ----
            chunks = [(t, nh, m)
                      for t in range(6) for nh in range(2) for m in range(MT)]

            # deferred actions: (due_chunk_index, fn); fill work for PE slack
            events = []
            fills = deque()
            # remaining pair-0 qk quarters, interleaved with the first chunks
            for slot, (tt, q) in enumerate([(6, 1), (6, 2), (6, 3)]):
                fills.append((slot, lambda tt=tt, q=q: qk_quarter(tt, q)))
            fills.append((5, lambda: qk_quarter(0, 2)))
            fills.append((6, lambda: qk_quarter(0, 3)))
            # v tiles during pair 0, one half per slot (pair 0's PV batches
            # are deferred past them)
            for vi in range(o["pre_v"], 2 * MT):
                m, vh = vi // 2, vi % 2
                base = o["v0_base"] if vh == 0 else o["v1_base"]
                fills.append((base + m, lambda m=m, vh=vh: v_half(m, vh)))
            # qk tiles for pair t generated during pair t-1
            for tp in range(1, 6):
                qs = [(tp, q) for q in range(4)] + [(6 + tp, q) for q in range(4)]
                for idx, (tt, q) in enumerate(qs):
                    if tp == 1:
                        # deadline-staggered: q tiles 2,3 and late k quarters
                        # aren't read until pair-1 nh1 / later m-chunks, so
                        # they move past the v-gen-congested slots
                        due = o["q1_dues"][idx]
                    else:
                        due = (tp - 1) * 16 + 2 * idx
                    fills.append((due, lambda tt=tt, q=q: qk_quarter(tt, q)))
            fills = deque(sorted(fills, key=lambda f: f[0]))

            if o["proj_split"]:
                for idx, (m, ph) in enumerate(
                        [(m, ph) for m in range(4, MT) for ph in range(2)]):
                    fills.append((o["part_base"] + idx,
                                  lambda m=m, ph=ph: proj_partial(m, ph)))
            fills = deque(sorted(fills, key=lambda f: f[0]))

            # initial generation: just enough for the first S chunks
            qk_quarter(0, 0)
            qk_quarter(0, 1)
            qk_quarter(6, 0)
            for pv_i in range(o["pre_v"]):
                v_half(pv_i // 2, pv_i % 2)

            n_chunks = len(chunks)
            for i in range(n_chunks + LAG):
                if i < n_chunks:
                    t, nh, m = chunks[i]
                    if (t, nh, m) == (4, 0, 0):
                        nc.sync.dma_start(
                            wp_all[:].rearrange("p (k n) -> p k n", k=KT),
                            wproj_d[:, :].rearrange("(k p) n -> p k n", p=128))
                    S_exp(i, t, nh, m)
                j = i - LAG
                if j >= 0:
                    tj, nhj, mj = chunks[j]
                    if mj == MT - 1:
                        due = i
                        if (tj, nhj) == (0, 0):
                            due = max(i, o["pv0_slot"])
                        elif (tj, nhj) == (0, 1):
                            due = max(i, o["pv0_slot"] + o["pv01_off"])
                        offs = o["end_offs"] if tj == 5 else o["mid_offs"]
                        for off, (jj, cs) in zip(offs, [(0, [0, 1]), (0, [2, 3]),
                                                        (1, [0, 1]), (1, [2, 3])]):
                            events.append(
                                (due + off,
                                 lambda tj=tj, nhj=nhj, jj=jj, cs=cs:
                                     pv_piece(tj, nhj, jj, cs)))
                        events.append((due + offs[4],
                                       lambda tj=tj, nhj=nhj: den_finish(tj, nhj)))
                        events.append((due + offs[5],
                                       lambda tj=tj, nhj=nhj: den_norm(tj, nhj)))
                        if not (tj == 5 and nhj == 1):
                            events.append((due + offs[6],
                                           lambda tj=tj, nhj=nhj: tr_nh(tj, nhj)))
                        if (tj, nhj) == (5, 0):
                            # proj of m-chunks 0..3 only needs pair-5 nh0
                            for pm in range(4):
                                events.append((due + offs[6] + 1,
                                               lambda pm=pm: proj(pm)))

                # run due deferred actions, then due fill items
                for ev in [e for e in events if e[0] <= i]:
                    events.remove(ev)
                    ev[1]()
                while fills and fills[0][0] <= i:
                    fills.popleft()[1]()

            for ev in sorted(events, key=lambda e: e[0]):
                ev[1]()
            tr_nh(5, 1)
            for m in range(4, MT):
                if o["proj_split"]:
                    proj_fin(m)
                else:
                    proj(m)

    nc.compile()
    return nc


def _pair_image(a, cols):
    """[768, cols] fp32 -> fp8 hi/lo SBUF images [128, 3*2*cols]
    with feature = 256*s + 128*i + p."""
    import ml_dtypes
    f8 = ml_dtypes.float8_e4m3
    hi = a.astype(f8)
    lo = (a - hi.astype(np.float32)).astype(f8)
    out = []
    for img in (hi, lo):
        out.append(np.ascontiguousarray(
            img.reshape(3, 2, 128, cols).transpose(2, 0, 1, 3).reshape(
                128, 3 * 2 * cols)))
    return out


def _run(inputs, trace=False, trace_kwargs=None):
    global _cached_nc
    import ml_dtypes
    bf16 = ml_dtypes.bfloat16
    x = np.asarray(inputs["x"], dtype=np.float32)
    wqkv = np.asarray(inputs["W_qkv"], dtype=np.float32)
    wproj = (np.asarray(inputs["W_proj"], dtype=np.float32) / 128.0).astype(bf16)
    whh, whl = _pair_image(wqkv[:, :2 * HID] * 32.0, 1536)
    wvh, wvl = _pair_image(wqkv[:, 2 * HID:] * 32.0, HID)
    ident = np.zeros((128, 129), dtype=bf16)
    ident[:, :128] = np.eye(128, dtype=np.float32).astype(bf16)
    ident[:, 128] = bf16(1.0)

    if _cached_nc is None:
        _cached_nc = build_program()
    nc = _cached_nc

    in_maps = []
    for b in range(B):
        xh, xl = _pair_image(np.ascontiguousarray(x[b].T) * 4.0, N)
        in_maps.append({"xh": xh, "xl": xl, "whh": whh, "whl": whl,
                        "wvh": wvh, "wvl": wvl, "wproj": wproj,
                        "ident": ident})
    kwargs = {}
    if trace:
        kwargs["trace"] = True
        if trace_kwargs:
            kwargs.update(trace_kwargs)
    try:
        res = run_bass_kernel_spmd(nc, in_maps, core_ids=list(range(B)), **kwargs)
    except Exception:
        # transient axon/PJRT hiccups happen; one retry
        res = run_bass_kernel_spmd(nc, in_maps, core_ids=list(range(B)), **kwargs)
    out = np.stack([np.asarray(r["y"], dtype=np.float32) for r in res.results],
                   axis=0)
    return out, res


def kernel(**inputs):
    out, _ = _run(inputs)
    return out
